# revision 16
# baseline (speedup 1.0000x reference)
"""Based-attention (Taylor linear attention + sliding window) TRN2 kernel.

Math: phi(u) = [1, u, outer(u,u)*sqrt(1/2)] satisfies
    phi(q) . phi(k) = 1 + q.k + 0.5*(q.k)^2
so causal linear attention with Taylor features is ordinary causal
attention with elementwise weights A = 0.5*(G+1)^2 + 0.5, G = Q @ K^T.
num/den are both linear in A, so the global 0.5 cancels: we use
A' = (G+1)^2 + 1.  The sliding-window softmax reuses the same G.

Sharding: H=16 heads over 8 cores (2 heads/core).  All inputs are cast
to fp16 on the host (full-rate PE, FWL weight loads, half DMA bytes).
Q and K projections are packed into one stationary [128, 96] weight so
one pass over x produces both; K is then partition-shifted 48->0 with a
small SBUF->SBUF DMA.  A PE warm-up spin at kernel start lifts the HAM
clock gate to 8/8 before the first real matmul.
"""

import sys

import numpy as np

sys.path.insert(0, "/opt/trn_rl_repo")

from concourse import bacc, mybir, tile  # noqa: E402
from concourse.bass_utils import run_bass_kernel_spmd  # noqa: E402

N = 1024
D = 1024
H = 16
DP = 16
DH = 64
W = 64
NCORES = 8
HPC = H // NCORES  # heads per core = 2

F32 = mybir.dt.float32
F16 = mybir.dt.float16

KT = D // 128  # 8 contraction tiles
NCH = N // 128  # 8 token chunks
GRP = 4  # query chunks per group
NG = NCH // GRP

N_WARMUP = 24  # PE spin matmuls to lift the HAM clock gate

# const tile layout (fp16 [128, CN]): [mlin 128 | mwin 192 | ident 128]
C_MLIN = 0
C_MWIN = 128
C_IDENT = 320
CN = 448

_CACHE = {}


def _emit(tc, nc, t):
    AluAdd = mybir.AluOpType.add
    AluMult = mybir.AluOpType.mult
    Act = mybir.ActivationFunctionType

    from contextlib import ExitStack

    with ExitStack() as ctx:
        cp = ctx.enter_context(tc.tile_pool(name="consts", bufs=1))

        # ---- PE warm-up spin (no deps; drains while DMAs land) ----
        wu_sb = cp.tile([128, 128], F16, tag="wu", name="wu")
        nc.gpsimd.memset(wu_sb[:], 0.0)
        with tc.tile_pool(name="pswu", bufs=1, space="PSUM") as pswu:
            wu_ps = pswu.tile([128, 128], F32, tag="wups", name="wups")
            for _ in range(N_WARMUP):
                nc.tensor.matmul(wu_ps[:], wu_sb[:], wu_sb[:], start=True, stop=True)

        # ---- input DMAs ----
        wqk_sb = cp.tile([128, KT * 96], F16, tag="wqk", name="wqk")
        nc.scalar.dma_start(wqk_sb[:], t["wqk"][:, :])
        wv_sb = cp.tile([128, KT * 128], F16, tag="wv", name="wv")
        nc.scalar.dma_start(wv_sb[:], t["wv"][:, :])
        csts = cp.tile([128, CN], F16, tag="csts", name="csts")
        nc.gpsimd.dma_start(csts[:], t["csts"][:, :])
        bias2 = cp.tile([128, 2], F32, tag="bias2", name="bias2")
        nc.gpsimd.dma_start(bias2[:], t["bias2"][:, :])
        onescol = cp.tile([128, 1], F16, tag="onescol", name="onescol")
        nc.gpsimd.memset(onescol[:], 1.0)
        ones_row = cp.tile([1, 128], F16, tag="ones_row", name="ones_row")
        nc.gpsimd.memset(ones_row[:], 1.0)

        xt = cp.tile([128, KT * N], F16, tag="xt", name="xt")
        for k in range(KT):
            eng = nc.sync if k % 2 == 0 else nc.scalar
            eng.dma_start(xt[:, N * k : N * k + N], t["xt"][:, N * k : N * k + N])

        mlin = csts[:, C_MLIN : C_MLIN + 128]
        mwin = csts[:, C_MWIN : C_MWIN + 192]
        ident = csts[:, C_IDENT : C_IDENT + 128]

        qkt = cp.tile([96, N], F16, tag="qkt", name="qkt")  # Q rows 0:48 (+bias)
        kt_sb = cp.tile([48, N], F16, tag="kt", name="kt")  # K shifted to base 0
        vt16 = cp.tile([128, N], F16, tag="vt", name="vt")
        # token-major V: per chunk [h0 64 | 1 | h1 64 | 1]
        vcat = cp.tile([128, NCH * 130], F16, tag="vcat", name="vcat")
        nc.gpsimd.memset(vcat[:], 1.0)
        ci_sb = cp.tile([1, NCH * 130], F16, tag="ci", name="ci")

        # ---- phase A: projections (fp16, QK packed) ----
        with tc.tile_pool(name="psA", bufs=1, space="PSUM") as psA, tc.tile_pool(
            name="psAt", bufs=2, space="PSUM"
        ) as psAt:
            psqk = psA.tile([96, N], F32, tag="psqk", name="psqk")
            psv = psA.tile([128, N], F32, tag="psv", name="psv")
            for half in range(2):
                s = slice(512 * half, 512 * half + 512)
                for k in range(KT):
                    nc.tensor.matmul(
                        psqk[:, s],
                        wqk_sb[:, 96 * k : 96 * k + 96],
                        xt[:, N * k + 512 * half : N * k + 512 * half + 512],
                        start=(k == 0),
                        stop=(k == KT - 1),
                    )
                for k in range(KT):
                    nc.tensor.matmul(
                        psv[:, s],
                        wv_sb[:, 128 * k : 128 * k + 128],
                        xt[:, N * k + 512 * half : N * k + 512 * half + 512],
                        start=(k == 0),
                        stop=(k == KT - 1),
                    )
                # bias + fp16 cast; Q and K in one pass (K at rows 48:96)
                nc.scalar.activation(
                    qkt[:, s], psqk[:, s], Act.Identity, bias=bias2[0:96, 0:1]
                )
                nc.scalar.activation(
                    vt16[:, s], psv[:, s], Act.Identity, bias=bias2[:, 1:2]
                )
                # shift K rows 48:96 -> partitions 0:48
                nc.sync.dma_start(kt_sb[:, s], qkt[48:96, s])
                # V token-major transposes for this half (fp16, PE)
                for c in range(4 * half, 4 * half + 4):
                    pst = psAt.tile([128, 128], F16, tag="vtr", name="vtr")
                    nc.tensor.transpose(
                        pst[:], vt16[:, 128 * c : 128 * c + 128], ident
                    )
                    nc.vector.tensor_copy(
                        vcat[:, 130 * c : 130 * c + 64], pst[:, 0:64]
                    )
                    nc.vector.tensor_copy(
                        vcat[:, 130 * c + 65 : 130 * c + 129], pst[:, 64:128]
                    )

            # ci[c] = colsum of vcat chunks 0..c (the "+1" off-diag constant
            # of A' = (G+1)^2 + 1, folded into a rank-1 row per chunk)
            for c in range(NCH):
                psc = psAt.tile([1, 130], F32, tag="psc", name="psc")
                nc.tensor.matmul(
                    psc[:],
                    onescol[:],
                    vcat[:, 130 * c : 130 * c + 130],
                    start=True,
                    stop=True,
                )
                if c == 0:
                    nc.vector.tensor_copy(ci_sb[:, 0:130], psc[:])
                else:
                    nc.vector.tensor_add(
                        ci_sb[:, 130 * c : 130 * c + 130],
                        ci_sb[:, 130 * c - 130 : 130 * c],
                        psc[:],
                    )

        if "dbg_qt" in t:
            nc.sync.dma_start(t["dbg_qt"][:, :], qkt[0:48, :])
            nc.sync.dma_start(t["dbg_kt"][:, :], kt_sb[:])
            nc.sync.dma_start(t["dbg_vt"][:, :], vt16[:])
            nc.sync.dma_start(t["dbg_vc"][:, :], vcat[:, 0:260])
            nc.sync.dma_start(t["dbg_ci"][:, :], ci_sb[:, :])

        # ---- phase B: attention, both heads together ----
        psg = ctx.enter_context(tc.tile_pool(name="psg", bufs=2, space="PSUM"))
        psy = ctx.enter_context(tc.tile_pool(name="psy", bufs=1, space="PSUM"))
        sba = ctx.enter_context(tc.tile_pool(name="sba", bufs=2))
        sbe = ctx.enter_context(tc.tile_pool(name="sbe", bufs=2))
        sbp = ctx.enter_context(tc.tile_pool(name="sbp", bufs=4))
        sbg1 = ctx.enter_context(tc.tile_pool(name="sbg1", bufs=2))

        # ACT handles Square for these j (per group); DVE 2-pass the rest.
        ACT_J = {0: (0, 1, 3), 1: (0, 1, 2, 6, 7)}

        for g in range(NG):
            i0, i1 = GRP * g, GRP * g + GRP
            m0, m1 = 128 * i0, 128 * i1
            # ys: one 4-bank tile; chunk i at cols 512(i-i0), per chunk:
            # [h0 lin 65 | h0 win 65 | h1 lin 65 | h1 win 65] = 260 cols
            ys = psy.tile([128, 2048], F32, tag="ys", name="ys")

            def yb(i, h, part):  # part 0=lin 1=win
                return 512 * (i - i0) + 130 * h + 65 * part

            # ci injection: first write per bank (start=True on h0)
            for i in range(max(i0, 1), i1):
                for h in range(HPC):
                    nc.tensor.matmul(
                        ys[:, yb(i, h, 0) : yb(i, h, 0) + 65],
                        ones_row[:],
                        ci_sb[0:1, 130 * (i - 1) + 65 * h : 130 * (i - 1) + 65 * h + 65],
                        start=(h == 0),
                        stop=False,
                        skip_group_check=True,
                    )

            yst = sbp.tile([128, 512], F32, tag="yst", name="yst")

            def norm_chunk(i):
                # chunk i's ys bank is complete; normalize + stage output now
                # so its PSUM region frees early (keeps PE fed across groups)
                q = 512 * (i - i0)
                rr = sbp.tile([128, 4], F32, tag="rr", name="rr")
                dens = (
                    ys[:, q + 64 : q + 64 + 260]
                    .rearrange("p (d c) -> p d c", d=4)[:, :, 0]
                )
                nc.vector.reciprocal(rr[:], dens)
                for h in range(HPC):
                    oc = 128 * (i - i0) + 64 * h
                    t1 = sbp.tile([128, 64], F32, tag="t1", name="t1")
                    nc.scalar.activation(
                        t1[:],
                        ys[:, yb(i, h, 0) : yb(i, h, 0) + 64],
                        Act.Identity,
                        scale=rr[:, 2 * h : 2 * h + 1],
                    )
                    nc.vector.scalar_tensor_tensor(
                        yst[:, oc : oc + 64],
                        ys[:, yb(i, h, 1) : yb(i, h, 1) + 64],
                        rr[:, 2 * h + 1 : 2 * h + 2],
                        t1[:],
                        AluMult,
                        AluAdd,
                    )

            for j in range(i1):
                mstart = max(128 * j, m0)
                span = m1 - mstart
                off = mstart - m0
                # G for both heads: h0 rows 0:16 (row-group 0), h1 rows
                # 32:48 (row-group 32) -> concurrent on the PE array.
                pg = psg.tile([128, 1024], F32, tag="pg", name="pg")
                for h in range(HPC):
                    r = slice(32 * h, 32 * h + 16)
                    nc.tensor.matmul(
                        pg[:, 512 * h + off : 512 * h + off + span],
                        kt_sb[r, 128 * j : 128 * j + 128],
                        qkt[r, mstart:m1],
                        start=True,
                        stop=True,
                    )
                pgpair = pg[:].rearrange("p (h q) -> p h q", h=2)[
                    :, :, off : off + span
                ]
                # A' = (G+1)^2 (+1 via ci/diag): ACT 1-pass or DVE 2-pass
                a = sba.tile([128, 1024], F16, tag="a", name="a")
                apair = a[:].rearrange("p (h q) -> p h q", h=2)[:, :, off : off + span]
                if j in ACT_J[g]:
                    nc.scalar.activation(apair, pgpair, Act.Square, bias=1.0)
                else:
                    g1 = sbg1.tile([128, 1024], F16, tag="g1", name="g1")
                    g1pair = g1[:].rearrange("p (h q) -> p h q", h=2)[
                        :, :, off : off + span
                    ]
                    nc.vector.tensor_scalar_add(g1pair, pgpair, 1.0)
                    nc.vector.tensor_mul(apair, g1pair, g1pair)
                if j >= i0:
                    dc = 128 * j - mstart
                    for h in range(HPC):
                        asl = a[:, 512 * h + off + dc : 512 * h + off + dc + 128]
                        nc.vector.scalar_tensor_tensor(
                            asl, asl, 1.0, mlin, AluAdd, AluMult
                        )
                # linear-attention matmuls (token-major: A slice stationary)
                for i in range(max(j, i0), i1):
                    ic = 128 * i - mstart
                    for h in range(HPC):
                        nc.tensor.matmul(
                            ys[:, yb(i, h, 0) : yb(i, h, 0) + 65],
                            a[:, 512 * h + off + ic : 512 * h + off + ic + 128],
                            vcat[:, 130 * j + 65 * h : 130 * j + 65 * h + 65],
                            start=(j == 0 and i == 0 and h == 0),
                            stop=(j == i),
                            skip_group_check=True,
                        )
                # sliding window: exp of G on [diag 128 | next 64]
                wlo = max(128 * j, m0)
                whi = min(128 * j + 192, m1)
                if whi > wlo:
                    ew = whi - wlo
                    e = sbe.tile([128, 384], F16, tag="e", name="e")
                    epair = e[:].rearrange("p (h q) -> p h q", h=2)[:, :, 0:ew]
                    pgw = pg[:].rearrange("p (h q) -> p h q", h=2)[
                        :, :, wlo - m0 : wlo - m0 + ew
                    ]
                    nc.scalar.activation(epair, pgw, Act.Exp)
                    mw0 = 0 if wlo == 128 * j else 128
                    for h in range(HPC):
                        esl = e[:, 192 * h : 192 * h + ew]
                        eng = nc.vector if h == 0 else nc.gpsimd
                        eng.tensor_mul(esl, esl, mwin[:, mw0 : mw0 + ew])
                    if wlo == 128 * j:  # diag part -> queries chunk j (last
                        # write to that win region -> stop)
                        for h in range(HPC):
                            nc.tensor.matmul(
                                ys[:, yb(j, h, 1) : yb(j, h, 1) + 65],
                                e[:, 192 * h : 192 * h + 128],
                                vcat[:, 130 * j + 65 * h : 130 * j + 65 * h + 65],
                                start=False,
                                stop=True,
                                skip_group_check=True,
                            )
                    if whi == 128 * j + 192:  # next-chunk part (64 cols)
                        ec = 128 if mw0 == 0 else 0
                        for h in range(HPC):
                            nc.tensor.matmul(
                                ys[0:64, yb(j + 1, h, 1) : yb(j + 1, h, 1) + 65],
                                e[:, 192 * h + ec : 192 * h + ec + 64],
                                vcat[:, 130 * j + 65 * h : 130 * j + 65 * h + 65],
                                start=False,
                                stop=False,
                                skip_group_check=True,
                            )
                if j >= i0:
                    norm_chunk(j)

            if "dbg_ys0" in t and g == 0:
                d0 = sbp.tile([128, 260], F32, tag="d0", name="d0")
                nc.vector.tensor_copy(d0[:], ys[:, 0:260])
                nc.sync.dma_start(t["dbg_ys0"][:, :], d0[:])

            # one output DMA per group: yst[p, (i h c)] -> y[512g+128i+p, 64h+c]
            yout = (
                t["y"][512 * g : 512 * g + 512, :]
                .rearrange("(i p) f -> p i f", p=128)
                .rearrange("p i (h c) -> p i h c", h=2)
            )
            nc.sync.dma_start(
                yout, yst[:].rearrange("p (i h c) -> p i h c", i=4, h=2)
            )


def _build(dbg=False):
    key = ("nc", dbg)
    if key in _CACHE:
        return _CACHE[key]
    nc = bacc.Bacc("TRN2", target_bir_lowering=False, debug=False)
    t = {
        "xt": nc.dram_tensor("xt", [128, KT * N], F16, kind="ExternalInput").ap(),
        "wqk": nc.dram_tensor("wqk", [128, KT * 96], F16, kind="ExternalInput").ap(),
        "wv": nc.dram_tensor("wv", [128, KT * 128], F16, kind="ExternalInput").ap(),
        "bias2": nc.dram_tensor("bias2", [128, 2], F32, kind="ExternalInput").ap(),
        "csts": nc.dram_tensor("csts", [128, CN], F16, kind="ExternalInput").ap(),
        "y": nc.dram_tensor("y", [N, HPC * DH], F32, kind="ExternalOutput").ap(),
    }
    if dbg:
        t["dbg_qt"] = nc.dram_tensor("dbg_qt", [48, N], F16, kind="ExternalOutput").ap()
        t["dbg_kt"] = nc.dram_tensor("dbg_kt", [48, N], F16, kind="ExternalOutput").ap()
        t["dbg_vt"] = nc.dram_tensor("dbg_vt", [128, N], F16, kind="ExternalOutput").ap()
        t["dbg_vc"] = nc.dram_tensor("dbg_vc", [128, 260], F16, kind="ExternalOutput").ap()
        t["dbg_ci"] = nc.dram_tensor("dbg_ci", [1, NCH * 130], F16, kind="ExternalOutput").ap()
        t["dbg_ys0"] = nc.dram_tensor("dbg_ys0", [128, 260], F32, kind="ExternalOutput").ap()
    with tile.TileContext(nc) as tc:
        _emit(tc, nc, t)
    nc.compile()
    _CACHE[key] = nc
    return nc


def _consts():
    n = np.arange(128)[:, None]
    m = np.arange(128)[None, :]
    mlin = (n <= m).astype(np.float16)
    mdiag = ((m - n >= 0) & (m - n <= W - 1)).astype(np.float16)
    mprev64 = (n[:, :] >= np.arange(64)[None, :] + W + 1).astype(np.float16)
    ident = np.eye(128, dtype=np.float16)
    out = np.zeros((128, CN), np.float16)
    out[:, C_MLIN : C_MLIN + 128] = mlin
    out[:, C_MWIN : C_MWIN + 128] = mdiag
    out[:, C_MWIN + 128 : C_MWIN + 192] = mprev64
    out[:, C_IDENT : C_IDENT + 128] = ident
    return out


def _pad48(w16x2):
    # [2,16,X] -> [48,X] with rows 0:16 = head0, 32:48 = head1
    out = np.zeros((48,) + w16x2.shape[2:], np.float32)
    out[0:16] = w16x2[0]
    out[32:48] = w16x2[1]
    return out


def _tile_kmajor(w, cols):
    # [D, cols] -> [128, KT*cols] fp16: tile k at cols [cols*k, cols*k+cols)
    out = np.empty((128, KT * cols), np.float16)
    for k in range(KT):
        out[:, cols * k : cols * k + cols] = w[128 * k : 128 * k + 128]
    return out


def _in_maps(x, Wq, bq, Wk, bk, Wv, bv):
    xs = np.asarray(x, np.float32)[0]  # [N, D]
    xT = np.ascontiguousarray(xs.T)
    csts = _consts()
    Wq = np.asarray(Wq, np.float32).reshape(H, DP, D)
    Wk = np.asarray(Wk, np.float32).reshape(H, DP, D)
    Wv = np.asarray(Wv, np.float32).reshape(H, DH, D)
    bq = np.asarray(bq, np.float32).reshape(H, DP)
    bk = np.asarray(bk, np.float32).reshape(H, DP)
    bv = np.asarray(bv, np.float32).reshape(H, DH)
    xt16 = _tile_kmajor(xT, N)
    maps = []
    for c in range(NCORES):
        hs = slice(HPC * c, HPC * c + HPC)
        wqkT = np.concatenate(
            [_pad48(Wq[hs]).T, _pad48(Wk[hs]).T], axis=1
        )  # [D, 96]
        wvT = np.ascontiguousarray(Wv[hs].reshape(HPC * DH, D).T)  # [D, 128]
        bias2 = np.zeros((128, 2), np.float32)
        bias2[0:48, 0] = _pad48(bq[hs])
        bias2[48:96, 0] = _pad48(bk[hs])
        bias2[:, 1] = bv[hs].reshape(HPC * DH)
        maps.append(
            {
                "xt": xt16,
                "wqk": _tile_kmajor(wqkT, 96),
                "wv": _tile_kmajor(wvT, 128),
                "bias2": bias2,
                "csts": csts,
            }
        )
    return maps


def _ensure_ntff_hook():
    """The agent image's antenv lacks axon_hooks; shim it so trace=True
    (NTFF profiling) works through bass_utils under axon."""
    import types

    try:
        import antenv.axon_hooks  # noqa: F401

        return
    except ImportError:
        pass
    try:
        import antenv
        from trn_agent_boot.trn_boot import _ntff_profile_via_ctypes

        hook = _ntff_profile_via_ctypes("/opt/axon/libaxon_pjrt.so")
        mod = types.ModuleType("antenv.axon_hooks")
        mod.get_axon_ntff_profile_hook = lambda: hook
        mod.set_axon_ntff_profile_hook = lambda h: None
        sys.modules["antenv.axon_hooks"] = mod
        antenv.axon_hooks = mod
    except Exception:
        pass


def _run(in_maps, trace=False, dbg=False):
    nc = _build(dbg)
    if trace:
        _ensure_ntff_hook()
    return run_bass_kernel_spmd(nc, in_maps, list(range(NCORES)), trace=trace)


def debug_run(x, Wq, bq, Wk, bk, Wv, bv):
    return _run(_in_maps(x, Wq, bq, Wk, bk, Wv, bv), dbg=True)


def kernel(x, Wq, bq, Wk, bk, Wv, bv):
    res = _run(_in_maps(x, Wq, bq, Wk, bk, Wv, bv))
    out = np.concatenate([res.results[c]["y"] for c in range(NCORES)], axis=1)
    return out[None].astype(np.float32)


def bench(x, Wq, bq, Wk, bk, Wv, bv):
    """Run with NTFF tracing; returns (output, exec_time_ns)."""
    res = _run(_in_maps(x, Wq, bq, Wk, bk, Wv, bv), trace=True)
    out = np.concatenate([res.results[c]["y"] for c in range(NCORES)], axis=1)
    return out[None].astype(np.float32), res.exec_time_ns


# revision 25
# speedup vs baseline: 1.2627x; 1.2627x over previous
"""Based-attention (Taylor linear attention + sliding window) TRN2 kernel.

Math: phi(u) = [1, u, outer(u,u)*sqrt(1/2)] satisfies
    phi(q) . phi(k) = 1 + q.k + 0.5*(q.k)^2
so causal linear attention with Taylor features is ordinary causal
attention with elementwise weights A = 0.5*(G+1)^2 + 0.5, G = Q @ K^T.
num/den are both linear in A, so the global 0.5 cancels: we use
A' = (G+1)^2 + 1.  The sliding-window softmax reuses the same G.

Sharding: H=16 heads over 8 cores (2 heads/core).  All inputs are cast
to fp16 on the host (full-rate PE, FWL weight loads, half DMA bytes).
Q and K projections are packed into one stationary [128, 96] weight so
one pass over x produces both; K is then partition-shifted 48->0 with a
small SBUF->SBUF DMA.  A PE warm-up spin at kernel start lifts the HAM
clock gate to 8/8 before the first real matmul.
"""

import sys

import numpy as np

sys.path.insert(0, "/opt/trn_rl_repo")

from concourse import bacc, mybir, tile  # noqa: E402
from concourse.bass_utils import run_bass_kernel_spmd  # noqa: E402

N = 1024
D = 1024
H = 16
DP = 16
DH = 64
W = 64
NCORES = 8
HPC = H // NCORES  # heads per core = 2

F32 = mybir.dt.float32
F16 = mybir.dt.float16

KT = D // 128  # 8 contraction tiles
NCH = N // 128  # 8 token chunks
GRP = 4  # query chunks per group
NG = NCH // GRP

N_WARMUP = 16  # PE spin matmuls to lift the HAM clock gate

# const tile layout (fp16 [128, CN]): [mlin 128 | mwin 192 | ident 128]
C_MLIN = 0
C_MWIN = 128
C_IDENT = 320
CN = 448

_CACHE = {}


def _emit(tc, nc, t):
    AluAdd = mybir.AluOpType.add
    AluMult = mybir.AluOpType.mult
    Act = mybir.ActivationFunctionType

    from contextlib import ExitStack

    with ExitStack() as ctx:
        cp = ctx.enter_context(tc.tile_pool(name="consts", bufs=1))

        # ---- PE warm-up spin (no deps; drains while DMAs land) ----
        wu_sb = cp.tile([128, 128], F16, tag="wu", name="wu")
        nc.vector.memset(wu_sb[:], 0.0)
        with tc.tile_pool(name="pswu", bufs=1, space="PSUM") as pswu:
            wu_ps = pswu.tile([128, 128], F32, tag="wups", name="wups")
            for _ in range(N_WARMUP):
                nc.tensor.matmul(wu_ps[:], wu_sb[:], wu_sb[:], start=True, stop=True)

        # ---- input DMAs ----
        wqk_sb = cp.tile([128, KT * 96], F16, tag="wqk", name="wqk")
        nc.scalar.dma_start(wqk_sb[:], t["wqk"][:, :])
        wv_sb = cp.tile([128, KT * 128], F16, tag="wv", name="wv")
        nc.scalar.dma_start(wv_sb[:], t["wv"][:, :])
        csts = cp.tile([128, CN], F16, tag="csts", name="csts")
        nc.gpsimd.dma_start(csts[:], t["csts"][:, :])
        bias2 = cp.tile([128, 2], F32, tag="bias2", name="bias2")
        nc.gpsimd.dma_start(bias2[:], t["bias2"][:, :])
        onescol = cp.tile([128, 1], F16, tag="onescol", name="onescol")
        nc.gpsimd.memset(onescol[:], 1.0)
        ones_row = cp.tile([1, 128], F16, tag="ones_row", name="ones_row")
        nc.gpsimd.memset(ones_row[:], 1.0)

        xt = cp.tile([128, KT * N], F16, tag="xt", name="xt")
        for k in range(KT):
            eng = nc.sync if k % 2 == 0 else nc.scalar
            eng.dma_start(xt[:, N * k : N * k + N], t["xt"][:, N * k : N * k + N])

        mlin = csts[:, C_MLIN : C_MLIN + 128]
        mwin = csts[:, C_MWIN : C_MWIN + 192]
        ident = csts[:, C_IDENT : C_IDENT + 128]

        qkt = cp.tile([96, N], F16, tag="qkt", name="qkt")  # Q rows 0:48 (+bias)
        kt_sb = cp.tile([48, N], F16, tag="kt", name="kt")  # K shifted to base 0
        vt16 = cp.tile([128, N], F16, tag="vt", name="vt")
        # token-major V: per chunk [h0 64 | 1 | h1 64 | 1]
        vcat = cp.tile([128, NCH * 130], F16, tag="vcat", name="vcat")
        nc.gpsimd.memset(vcat[:], 1.0)
        ci_sb = cp.tile([1, NCH * 130], F16, tag="ci", name="ci")

        # ---- phase A: projections (fp16, QK packed) ----
        with tc.tile_pool(name="psA", bufs=1, space="PSUM") as psA, tc.tile_pool(
            name="psAt", bufs=2, space="PSUM"
        ) as psAt:
            psqk = psA.tile([96, N], F32, tag="psqk", name="psqk")
            psv = psA.tile([128, N], F32, tag="psv", name="psv")
            for half in range(2):
                s = slice(512 * half, 512 * half + 512)
                for k in range(KT):
                    xsl = xt[:, N * k + 512 * half : N * k + 512 * half + 512]
                    nc.tensor.matmul(
                        psqk[:, s],
                        wqk_sb[:, 96 * k : 96 * k + 96],
                        xsl,
                        start=(k == 0),
                        stop=(k == KT - 1),
                    )
                    nc.tensor.matmul(
                        psv[:, s],
                        wv_sb[:, 128 * k : 128 * k + 128],
                        xsl,
                        start=(k == 0),
                        stop=(k == KT - 1),
                    )
                # bias + fp16 cast; Q and K in one pass (K at rows 48:96)
                nc.scalar.activation(
                    qkt[:, s], psqk[:, s], Act.Identity, bias=bias2[0:96, 0:1]
                )
                nc.scalar.activation(
                    vt16[:, s], psv[:, s], Act.Identity, bias=bias2[:, 1:2]
                )
                # shift K rows 48:96 -> partitions 0:48
                nc.sync.dma_start(kt_sb[:, s], qkt[48:96, s])
                # V token-major transposes for this half (fp16, PE)
                for c in range(4 * half, 4 * half + 4):
                    pst = psAt.tile([128, 128], F16, tag="vtr", name="vtr")
                    nc.tensor.transpose(
                        pst[:], vt16[:, 128 * c : 128 * c + 128], ident
                    )
                    nc.vector.tensor_copy(
                        vcat[:, 130 * c : 130 * c + 64], pst[:, 0:64]
                    )
                    nc.vector.tensor_copy(
                        vcat[:, 130 * c + 65 : 130 * c + 129], pst[:, 64:128]
                    )

            # ci[c] = colsum of vcat chunks 0..c (the "+1" off-diag constant
            # of A' = (G+1)^2 + 1, folded into a rank-1 row per chunk)
            for c in range(NCH):
                psc = psAt.tile([1, 130], F32, tag="psc", name="psc")
                nc.tensor.matmul(
                    psc[:],
                    onescol[:],
                    vcat[:, 130 * c : 130 * c + 130],
                    start=True,
                    stop=True,
                )
                if c == 0:
                    nc.vector.tensor_copy(ci_sb[:, 0:130], psc[:])
                else:
                    nc.vector.tensor_add(
                        ci_sb[:, 130 * c : 130 * c + 130],
                        ci_sb[:, 130 * c - 130 : 130 * c],
                        psc[:],
                    )

        if "dbg_qt" in t:
            nc.sync.dma_start(t["dbg_qt"][:, :], qkt[0:48, :])
            nc.sync.dma_start(t["dbg_kt"][:, :], kt_sb[:])
            nc.sync.dma_start(t["dbg_vt"][:, :], vt16[:])
            nc.sync.dma_start(t["dbg_vc"][:, :], vcat[:, 0:260])
            nc.sync.dma_start(t["dbg_ci"][:, :], ci_sb[:, :])

        # ---- phase B: attention, both heads together ----
        psg = ctx.enter_context(tc.tile_pool(name="psg", bufs=2, space="PSUM"))
        psy = ctx.enter_context(tc.tile_pool(name="psy", bufs=1, space="PSUM"))
        sba = ctx.enter_context(tc.tile_pool(name="sba", bufs=2))
        sbe = ctx.enter_context(tc.tile_pool(name="sbe", bufs=2))
        sbp = ctx.enter_context(tc.tile_pool(name="sbp", bufs=4))

        for g in range(NG):
            i0, i1 = GRP * g, GRP * g + GRP
            m0, m1 = 128 * i0, 128 * i1
            # ys: one 4-bank tile; chunk i at cols 512(i-i0), per chunk:
            # [h0 lin 65 | h0 win 65 | h1 lin 65 | h1 win 65] = 260 cols
            ys = psy.tile([128, 2048], F32, tag="ys", name="ys")

            def yb(i, h, part):  # part 0=lin 1=win
                return 512 * (i - i0) + 130 * h + 65 * part

            # ci injection: first write per bank (start=True on h0)
            for i in range(max(i0, 1), i1):
                for h in range(HPC):
                    nc.tensor.matmul(
                        ys[:, yb(i, h, 0) : yb(i, h, 0) + 65],
                        ones_row[:],
                        ci_sb[0:1, 130 * (i - 1) + 65 * h : 130 * (i - 1) + 65 * h + 65],
                        start=(h == 0),
                        stop=False,
                        skip_group_check=True,
                    )

            for j in range(i1):
                mstart = max(128 * j, m0)
                span = m1 - mstart
                off = mstart - m0
                # G for both heads: h0 rows 0:16 (row-group 0), h1 rows
                # 32:48 (row-group 32) -> concurrent on the PE array.
                pg = psg.tile([128, 1024], F32, tag="pg", name="pg")
                for h in range(HPC):
                    r = slice(32 * h, 32 * h + 16)
                    nc.tensor.matmul(
                        pg[:, 512 * h + off : 512 * h + off + span],
                        kt_sb[r, 128 * j : 128 * j + 128],
                        qkt[r, mstart:m1],
                        start=True,
                        stop=True,
                    )
                pgpair = pg[:].rearrange("p (h q) -> p h q", h=2)[
                    :, :, off : off + span
                ]
                # A' = (G+1)^2 (+1 via ci/diag)
                a = sba.tile([128, 1024], F16, tag="a", name="a")
                apair = a[:].rearrange("p (h q) -> p h q", h=2)[:, :, off : off + span]
                nc.scalar.activation(apair, pgpair, Act.Square, bias=1.0)
                if j >= i0:
                    dc = 128 * j - mstart
                    for h in range(HPC):
                        asl = a[:, 512 * h + off + dc : 512 * h + off + dc + 128]
                        nc.vector.scalar_tensor_tensor(
                            asl, asl, 1.0, mlin, AluAdd, AluMult
                        )
                # linear-attention matmuls (token-major: A slice stationary).
                # The i==j diagonal AV goes LAST: it additionally waits on the
                # DVE mask, so emitting it last keeps the PE queue moving.
                ivals = [i for i in range(max(j, i0), i1) if i != j]
                if j >= i0:
                    ivals.append(j)
                for i in ivals:
                    ic = 128 * i - mstart
                    for h in range(HPC):
                        nc.tensor.matmul(
                            ys[:, yb(i, h, 0) : yb(i, h, 0) + 65],
                            a[:, 512 * h + off + ic : 512 * h + off + ic + 128],
                            vcat[:, 130 * j + 65 * h : 130 * j + 65 * h + 65],
                            start=(j == 0 and i == 0 and h == 0),
                            stop=(j == i),
                            skip_group_check=True,
                        )
                # sliding window: exp of G on [diag 128 | next 64]
                wlo = max(128 * j, m0)
                whi = min(128 * j + 192, m1)
                if whi > wlo:
                    ew = whi - wlo
                    e = sbe.tile([128, 384], F16, tag="e", name="e")
                    epair = e[:].rearrange("p (h q) -> p h q", h=2)[:, :, 0:ew]
                    pgw = pg[:].rearrange("p (h q) -> p h q", h=2)[
                        :, :, wlo - m0 : wlo - m0 + ew
                    ]
                    nc.scalar.activation(epair, pgw, Act.Exp)
                    mw0 = 0 if wlo == 128 * j else 128
                    for h in range(HPC):
                        esl = e[:, 192 * h : 192 * h + ew]
                        nc.vector.tensor_mul(esl, esl, mwin[:, mw0 : mw0 + ew])
                    if wlo == 128 * j:  # diag part -> queries chunk j (last
                        # write to that win region -> stop)
                        for h in range(HPC):
                            nc.tensor.matmul(
                                ys[:, yb(j, h, 1) : yb(j, h, 1) + 65],
                                e[:, 192 * h : 192 * h + 128],
                                vcat[:, 130 * j + 65 * h : 130 * j + 65 * h + 65],
                                start=False,
                                stop=True,
                                skip_group_check=True,
                            )
                    if whi == 128 * j + 192:  # next-chunk part (64 cols)
                        ec = 128 if mw0 == 0 else 0
                        for h in range(HPC):
                            nc.tensor.matmul(
                                ys[0:64, yb(j + 1, h, 1) : yb(j + 1, h, 1) + 65],
                                e[:, 192 * h + ec : 192 * h + ec + 64],
                                vcat[:, 130 * j + 65 * h : 130 * j + 65 * h + 65],
                                start=False,
                                stop=False,
                                skip_group_check=True,
                            )
            if "dbg_ys0" in t and g == 0:
                d0 = sbp.tile([128, 260], F32, tag="d0", name="d0")
                nc.vector.tensor_copy(d0[:], ys[:, 0:260])
                nc.sync.dma_start(t["dbg_ys0"][:, :], d0[:])

            # ---- batched normalization (broadcast rr along the free dim) ----
            rr = sbp.tile([128, 16], F32, tag="rr", name="rr")
            dens = (
                ys[:]
                .rearrange("p (i r) -> p i r", i=4)[:, :, 64 : 64 + 260]
                .rearrange("p i (d c) -> p i d c", d=4)[:, :, :, 0]
            )
            nc.vector.reciprocal(rr[:], dens)
            rrv = rr[:].rearrange("p (i d) -> p i d", i=4)
            rrl = rrv[:, :, 0:4:2].broadcast_to([128, 4, 2, 64])
            rrw = rrv[:, :, 1:4:2].broadcast_to([128, 4, 2, 64])
            ysv = ys[:].rearrange("p (i r) -> p i r", i=4)
            lin4 = ysv[:, :, 0:260].rearrange("p i (h z) -> p i h z", h=2)[
                :, :, :, 0:64
            ]
            win4 = ysv[:, :, 0:260].rearrange("p i (h z) -> p i h z", h=2)[
                :, :, :, 65:129
            ]
            yst = sbp.tile([128, 512], F32, tag="yst", name="yst")
            tmp = sbp.tile([128, 512], F32, tag="tmp", name="tmp")
            ystv = yst[:].rearrange("p (i h c) -> p i h c", i=4, h=2)
            tmpv = tmp[:].rearrange("p (i h c) -> p i h c", i=4, h=2)
            nc.vector.tensor_tensor(ystv, lin4, rrl, AluMult)
            nc.vector.tensor_tensor(tmpv, win4, rrw, AluMult)
            nc.vector.tensor_add(yst[:], yst[:], tmp[:])
            # one output DMA per group: yst[p, (i h c)] -> y[512g+128i+p, 64h+c]
            yout = (
                t["y"][512 * g : 512 * g + 512, :]
                .rearrange("(i p) f -> p i f", p=128)
                .rearrange("p i (h c) -> p i h c", h=2)
            )
            nc.sync.dma_start(
                yout, yst[:].rearrange("p (i h c) -> p i h c", i=4, h=2)
            )


def _build(dbg=False):
    key = ("nc", dbg)
    if key in _CACHE:
        return _CACHE[key]
    nc = bacc.Bacc("TRN2", target_bir_lowering=False, debug=False)
    t = {
        "xt": nc.dram_tensor("xt", [128, KT * N], F16, kind="ExternalInput").ap(),
        "wqk": nc.dram_tensor("wqk", [128, KT * 96], F16, kind="ExternalInput").ap(),
        "wv": nc.dram_tensor("wv", [128, KT * 128], F16, kind="ExternalInput").ap(),
        "bias2": nc.dram_tensor("bias2", [128, 2], F32, kind="ExternalInput").ap(),
        "csts": nc.dram_tensor("csts", [128, CN], F16, kind="ExternalInput").ap(),
        "y": nc.dram_tensor("y", [N, HPC * DH], F32, kind="ExternalOutput").ap(),
    }
    if dbg:
        t["dbg_qt"] = nc.dram_tensor("dbg_qt", [48, N], F16, kind="ExternalOutput").ap()
        t["dbg_kt"] = nc.dram_tensor("dbg_kt", [48, N], F16, kind="ExternalOutput").ap()
        t["dbg_vt"] = nc.dram_tensor("dbg_vt", [128, N], F16, kind="ExternalOutput").ap()
        t["dbg_vc"] = nc.dram_tensor("dbg_vc", [128, 260], F16, kind="ExternalOutput").ap()
        t["dbg_ci"] = nc.dram_tensor("dbg_ci", [1, NCH * 130], F16, kind="ExternalOutput").ap()
        t["dbg_ys0"] = nc.dram_tensor("dbg_ys0", [128, 260], F32, kind="ExternalOutput").ap()
    with tile.TileContext(nc) as tc:
        _emit(tc, nc, t)
    nc.compile()
    _CACHE[key] = nc
    return nc


def _consts():
    n = np.arange(128)[:, None]
    m = np.arange(128)[None, :]
    mlin = (n <= m).astype(np.float16)
    mdiag = ((m - n >= 0) & (m - n <= W - 1)).astype(np.float16)
    mprev64 = (n[:, :] >= np.arange(64)[None, :] + W + 1).astype(np.float16)
    ident = np.eye(128, dtype=np.float16)
    out = np.zeros((128, CN), np.float16)
    out[:, C_MLIN : C_MLIN + 128] = mlin
    out[:, C_MWIN : C_MWIN + 128] = mdiag
    out[:, C_MWIN + 128 : C_MWIN + 192] = mprev64
    out[:, C_IDENT : C_IDENT + 128] = ident
    return out


def _pad48(w16x2):
    # [2,16,X] -> [48,X] with rows 0:16 = head0, 32:48 = head1
    out = np.zeros((48,) + w16x2.shape[2:], np.float32)
    out[0:16] = w16x2[0]
    out[32:48] = w16x2[1]
    return out


def _tile_kmajor(w, cols):
    # [D, cols] -> [128, KT*cols] fp16: tile k at cols [cols*k, cols*k+cols)
    out = np.empty((128, KT * cols), np.float16)
    for k in range(KT):
        out[:, cols * k : cols * k + cols] = w[128 * k : 128 * k + 128]
    return out


def _in_maps(x, Wq, bq, Wk, bk, Wv, bv):
    xs = np.asarray(x, np.float32)[0]  # [N, D]
    xT = np.ascontiguousarray(xs.T)
    csts = _consts()
    Wq = np.asarray(Wq, np.float32).reshape(H, DP, D)
    Wk = np.asarray(Wk, np.float32).reshape(H, DP, D)
    Wv = np.asarray(Wv, np.float32).reshape(H, DH, D)
    bq = np.asarray(bq, np.float32).reshape(H, DP)
    bk = np.asarray(bk, np.float32).reshape(H, DP)
    bv = np.asarray(bv, np.float32).reshape(H, DH)
    xt16 = _tile_kmajor(xT, N)
    maps = []
    for c in range(NCORES):
        hs = slice(HPC * c, HPC * c + HPC)
        wqkT = np.concatenate(
            [_pad48(Wq[hs]).T, _pad48(Wk[hs]).T], axis=1
        )  # [D, 96]
        wvT = np.ascontiguousarray(Wv[hs].reshape(HPC * DH, D).T)  # [D, 128]
        bias2 = np.zeros((128, 2), np.float32)
        bias2[0:48, 0] = _pad48(bq[hs])
        bias2[48:96, 0] = _pad48(bk[hs])
        bias2[:, 1] = bv[hs].reshape(HPC * DH)
        maps.append(
            {
                "xt": xt16,
                "wqk": _tile_kmajor(wqkT, 96),
                "wv": _tile_kmajor(wvT, 128),
                "bias2": bias2,
                "csts": csts,
            }
        )
    return maps


def _ensure_ntff_hook():
    """The agent image's antenv lacks axon_hooks; shim it so trace=True
    (NTFF profiling) works through bass_utils under axon."""
    import types

    try:
        import antenv.axon_hooks  # noqa: F401

        return
    except ImportError:
        pass
    try:
        import antenv
        from trn_agent_boot.trn_boot import _ntff_profile_via_ctypes

        hook = _ntff_profile_via_ctypes("/opt/axon/libaxon_pjrt.so")
        mod = types.ModuleType("antenv.axon_hooks")
        mod.get_axon_ntff_profile_hook = lambda: hook
        mod.set_axon_ntff_profile_hook = lambda h: None
        sys.modules["antenv.axon_hooks"] = mod
        antenv.axon_hooks = mod
    except Exception:
        pass


def _run(in_maps, trace=False, dbg=False):
    nc = _build(dbg)
    if trace:
        _ensure_ntff_hook()
    return run_bass_kernel_spmd(nc, in_maps, list(range(NCORES)), trace=trace)


def debug_run(x, Wq, bq, Wk, bk, Wv, bv):
    return _run(_in_maps(x, Wq, bq, Wk, bk, Wv, bv), dbg=True)


def kernel(x, Wq, bq, Wk, bk, Wv, bv):
    res = _run(_in_maps(x, Wq, bq, Wk, bk, Wv, bv))
    out = np.concatenate([res.results[c]["y"] for c in range(NCORES)], axis=1)
    return out[None].astype(np.float32)


def bench(x, Wq, bq, Wk, bk, Wv, bv):
    """Run with NTFF tracing; returns (output, exec_time_ns)."""
    res = _run(_in_maps(x, Wq, bq, Wk, bk, Wv, bv), trace=True)
    out = np.concatenate([res.results[c]["y"] for c in range(NCORES)], axis=1)
    return out[None].astype(np.float32), res.exec_time_ns
